# revision 29
# baseline (speedup 1.0000x reference)
"""MultiHeadAttention (B=4, S=1024, D=1024, H=16) on 8 TRN2 NeuronCores.

Sharding (no collectives): core i handles batch b=i//2 and query-row half
i%2 (512 query tokens). K/V projections for the batch are duplicated across
the two cores sharing it; each core computes its 512 output rows fully.

Layouts on device (all "transposed"/feature-major so the d_model contraction
sits on SBUF partitions):
  qT  [1024, 512]   q_in[b, rows].T  (host-transposed, bf16)
  kT  [1024, 1024]  k_in[b].T        (f16)
  vT  [1024, 1024]  v_in[b].T        (f16)
  W*  [1024, 1024]  natural [d_in, d_out] (f16)
  outT[1024, 512]   fp32 -> host transposes back

Per-head attention with scores kept transposed (Sk on partitions, Sq free)
so softmax(P)@V needs no on-chip transposes; the softmax denominator comes
for free from a ones-column appended to each head's V block; normalization
is folded in post-V (1/denom commutes with the V matmul per query column).

Matmul operands are bf16 (fp32 weights would serialize a two-pass
LDWEIGHTS against every matmul: measured 396ns/MM vs 213ns streaming);
accumulation and the softmax denominator chain stay fp32 in PSUM.
All PSUM tiles are [128, 1024] two-bank pairs (4 bufs = all 8 banks); exps
run one [128, 1024] activation per head pair; attention PSUM is drained to
SBUF by a short DVE copy so the banks free before the normalization chain.
"""

import sys

sys.path.insert(0, "/opt/trn_rl_repo")

import numpy as np

D = 1024
H = 16
DH = 64
P = 128
SQ = 512      # query tokens per core
S = 1024      # kv tokens per core (full batch)
NT = 8        # number of 128-wide tiles along d_model
N_CORES = 8

_CACHE = {}
TRACE = False  # set True (e.g. from test.py) to capture an NTFF profile
TMPDIR = None  # where to keep NEFF/NTFF artifacts when tracing


def _build():
    import concourse.bacc as bacc
    import concourse.mybir as mybir
    import concourse.tile as tile

    f32 = mybir.dt.float32
    f16 = mybir.dt.float16
    AF = mybir.ActivationFunctionType

    nc = bacc.Bacc("TRN2", target_bir_lowering=False, debug=False, num_devices=N_CORES)

    qT_d = nc.dram_tensor("qT", [D, SQ], f16, kind="ExternalInput")
    kT_d = nc.dram_tensor("kT", [D, S], f16, kind="ExternalInput")
    vT_d = nc.dram_tensor("vT", [D, S], f16, kind="ExternalInput")
    Wq_d = nc.dram_tensor("Wq", [D, D], f16, kind="ExternalInput")
    Wk_d = nc.dram_tensor("Wk", [D, D], f16, kind="ExternalInput")
    Wv_d = nc.dram_tensor("Wv", [D, D], f16, kind="ExternalInput")
    Wo_d = nc.dram_tensor("Wo", [D, D], f16, kind="ExternalInput")
    bq_d = nc.dram_tensor("bq", [P, NT], f32, kind="ExternalInput")
    bk_d = nc.dram_tensor("bk", [P, NT], f32, kind="ExternalInput")
    bo_d = nc.dram_tensor("bo_eff", [P, NT], f32, kind="ExternalInput")
    ones_d = nc.dram_tensor("ones", [P, NT, H], f16, kind="ExternalInput")
    outT_d = nc.dram_tensor("outT", [D, SQ], f32, kind="ExternalOutput")

    with tile.TileContext(nc) as tc:
        with (
            tc.tile_pool(name="res", bufs=1) as res,
            tc.tile_pool(name="res2", bufs=1) as res2,
            tc.tile_pool(name="stream", bufs=6) as stream,
            tc.tile_pool(name="expp", bufs=6) as expp,
            tc.tile_pool(name="small", bufs=4) as small,
            tc.tile_pool(name="psum", bufs=4, space="PSUM") as psum,
        ):
            # ---- resident SBUF tensors ----
            QT_sb = res.tile([P, NT, SQ], f16, tag="QT")        # [d_part, m, sq]
            KT_sb = res.tile([P, NT, S], f16, tag="KT")         # [d_part, m, sk]
            Vaug_sb = res.tile([P, NT, H * (DH + 1)], f16, tag="Vaug")  # [sk_part, m, 16*65]
            attn_sb = res.tile([P, NT, SQ], f16, tag="attn")    # [d_part, kt, sq]
            bq_sb = res.tile([P, NT], f32, tag="bq")
            bk_sb = res.tile([P, NT], f32, tag="bk")
            bo_sb = res.tile([P, NT], f32, tag="bo")
            ones_sb = res.tile([P, NT, H], f16, tag="ones")

            # ---- Q projection: QT[128m+i, t] = sum_k Wq[k, 128m+i] qT[k, t] ----
            qps = {}
            for k in range(NT):
                w_s = stream.tile([P, D], f16, tag="wstrip")
                nc.sync.dma_start(w_s[:], Wq_d[P * k : P * (k + 1), :])
                x_s = stream.tile([P, SQ], f16, tag="xstrip")
                nc.sync.dma_start(x_s[:], qT_d[P * k : P * (k + 1), :])
                for m in range(NT):
                    if k == 0 and m % 2 == 0:
                        qps[m // 2] = psum.tile([P, 1024], f32, tag="ps", name=f"qps{m//2}")
                    nc.tensor.matmul(
                        qps[m // 2][:, 512 * (m % 2) : 512 * (m % 2) + 512],
                        lhsT=w_s[:, P * m : P * (m + 1)],
                        rhs=x_s[:],
                        start=(k == 0),
                        stop=(k == NT - 1),
                    )
            # small/scattered loads, emitted late so they don't head-block the
            # DMA queues ahead of the first weight strips
            nc.sync.dma_start(bq_sb[:], bq_d[:])
            nc.sync.dma_start(bk_sb[:], bk_d[:])
            nc.sync.dma_start(bo_sb[:], bo_d[:])
            nc.sync.dma_start(ones_sb[:], ones_d[:])
            for m in range(NT):
                nc.scalar.activation(
                    QT_sb[:, m, :],
                    qps[m // 2][:, 512 * (m % 2) : 512 * (m % 2) + 512],
                    AF.Identity,
                    bias=bq_sb[:, m : m + 1],
                )

            # ones columns of V_aug via DVE strided copy from the staged tile
            # (direct scatter-DMA costs ~16k 4B descriptors and clogs the queue)
            ones_cols = Vaug_sb[:].rearrange("p m (h c) -> p m h c", c=DH + 1)[
                :, :, :, DH
            ]
            nc.vector.tensor_copy(ones_cols, ones_sb[:])

            # ---- K projection ----
            # kT resident in SBUF; Wk streamed once as half-strips; one
            # N=1024 matmul per (k, m) into a two-bank pair tile.
            kT_res = res2.tile([P, NT, S], f16, tag="kT")
            for half in range(2):
                kps = {}
                for k in range(NT):
                    w_s = stream.tile([P, 512], f16, tag="xstrip", name=f"wkh{half}_{k}")
                    nc.sync.dma_start(
                        w_s[:], Wk_d[P * k : P * (k + 1), 512 * half : 512 * (half + 1)]
                    )
                    if half == 0:
                        nc.sync.dma_start(
                            kT_res[:, k, :], kT_d[P * k : P * (k + 1), :]
                        )
                    for mm in range(4):
                        m = 4 * half + mm
                        if k == 0:
                            kps[m] = psum.tile([P, 1024], f32, tag="ps", name=f"kps{m}")
                        for n in range(2):
                            nc.tensor.matmul(
                                kps[m][:, 512 * n : 512 * (n + 1)],
                                lhsT=w_s[:, P * mm : P * (mm + 1)],
                                rhs=kT_res[:, k, 512 * n : 512 * (n + 1)],
                                start=(k == 0),
                                stop=(k == NT - 1),
                            )
                for mm in range(4):
                    m = 4 * half + mm
                    nc.scalar.activation(
                        KT_sb[:, m, :], kps[m][:], AF.Identity, bias=bk_sb[:, m : m + 1]
                    )

            # ---- V projection (d-halves) + attention, interleaved so the
            # first four head pairs (d-half 0) start while V half 1 projects.
            def v_proj_half(n):
                vps = {}
                for k in range(NT):
                    v_s = stream.tile([P, S], f16, tag="wstrip", name=f"vs{n}_{k}")
                    nc.sync.dma_start(v_s[:], vT_d[P * k : P * (k + 1), :])
                    w_s = stream.tile([P, 512], f16, tag="xstrip", name=f"wvh{n}_{k}")
                    nc.sync.dma_start(
                        w_s[:], Wv_d[P * k : P * (k + 1), 512 * n : 512 * (n + 1)]
                    )
                    for m in range(NT):
                        if k == 0 and m % 2 == 0:
                            vps[m // 2] = psum.tile(
                                [P, 1024], f32, tag="ps", name=f"vps{n}_{m//2}"
                            )
                        nc.tensor.matmul(
                            vps[m // 2][:, 512 * (m % 2) : 512 * (m % 2) + 512],
                            lhsT=v_s[:, P * m : P * (m + 1)],
                            rhs=w_s[:],
                            start=(k == 0),
                            stop=(k == NT - 1),
                        )
                for m in range(NT):
                    dst = Vaug_sb[:, m, 520 * n : 520 * n + 520].rearrange(
                        "p (h c) -> p h c", c=DH + 1
                    )[:, :, 0:DH]
                    src = vps[m // 2][
                        :, 512 * (m % 2) : 512 * (m % 2) + 512
                    ].rearrange("p (h c) -> p h c", c=DH)
                    if m % 2 == 0:
                        nc.vector.tensor_copy(dst, src)
                    else:
                        nc.scalar.copy(dst, src)

            def attention_pair(p):
                h0, h1 = 2 * p, 2 * p + 1
                att = psum.tile([P, 1024], f32, tag="ps", name=f"att{p}")
                for s in range(NT):
                    sc = psum.tile([P, 1024], f32, tag="ps", name=f"sc{p}_{s}")
                    nc.tensor.matmul(
                        sc[:, 0:512],
                        lhsT=KT_sb[0:64, p, P * s : P * (s + 1)],
                        rhs=QT_sb[0:64, p, :],
                        start=True,
                        stop=True,
                    )
                    nc.tensor.matmul(
                        sc[:, 512:1024],
                        lhsT=KT_sb[64:128, p, P * s : P * (s + 1)],
                        rhs=QT_sb[64:128, p, :],
                        start=True,
                        stop=True,
                    )
                    e = expp.tile([P, 1024], f16, tag="exp", name=f"e{p}_{s}")
                    nc.scalar.activation(e[:], sc[:], AF.Exp, scale=0.125)
                    nc.tensor.matmul(
                        att[0 : DH + 1, 0:512],
                        lhsT=Vaug_sb[:, s, 65 * h0 : 65 * h0 + 65],
                        rhs=e[:, 0:512],
                        start=(s == 0),
                        stop=(s == NT - 1),
                    )
                    nc.tensor.matmul(
                        att[0 : DH + 1, 512:1024],
                        lhsT=Vaug_sb[:, s, 65 * h1 : 65 * h1 + 65],
                        rhs=e[:, 512:1024],
                        start=(s == 0),
                        stop=(s == NT - 1),
                    )
                # drain PSUM fast (frees both banks); normalization happens a
                # pair later (attention_norm) so this copy never queues behind
                # the previous pair's normalize chain on DVE.
                acp = small.tile([DH + 1, 1024], f32, tag="attcp", name=f"acp{p}")
                nc.vector.tensor_copy(acp[:], att[0 : DH + 1, :])
                return acp

            def attention_norm(p, acp):
                rc = small.tile([1, 1024], f32, tag="recip", name=f"rc{p}")
                nc.vector.reciprocal(rc[:], acp[DH : DH + 1, :])
                bc = small.tile([64, 1024], f32, tag="bc", name=f"bc{p}")
                nc.gpsimd.partition_broadcast(bc[:], rc[:])
                nc.gpsimd.tensor_mul(
                    attn_sb[0:64, p, :], acp[0:DH, 0:512], bc[:, 0:512]
                )
                t1 = small.tile([64, 512], f16, tag="tmp", name=f"t1{p}")
                nc.gpsimd.tensor_mul(t1[:], acp[0:DH, 512:1024], bc[:, 512:1024])
                nc.sync.dma_start(attn_sb[64:128, p, :], t1[:])

            pend = []  # (p, acp) pending normalization, one pair behind
            v_proj_half(0)
            for p in range(3):
                pend.append((p, attention_pair(p)))
                if len(pend) > 1:
                    attention_norm(*pend.pop(0))
            pend.append((3, attention_pair(3)))
            v_proj_half(1)  # Vaug half-1 copies go ahead of the pending norms
            for p in range(4, NT):
                attention_norm(*pend.pop(0))
                pend.append((p, attention_pair(p)))
                if len(pend) > 1:
                    attention_norm(*pend.pop(0))
            # prefetch the first output-projection weight strips while the last
            # pairs drain (only 5: a 6-slot pool fully claimed here would block
            # the sync queue ahead of the t1 DMAs the norms below still need)
            wo_strips = {}
            for kt in range(5):
                w_s = stream.tile([P, D], f16, tag="wstrip", name=f"wo{kt}")
                nc.sync.dma_start(w_s[:], Wo_d[P * kt : P * (kt + 1), :])
                wo_strips[kt] = w_s
            while pend:
                attention_norm(*pend.pop(0))

            # ---- output projection: outT[128m+i, t] = sum_d Wo[d, 128m+i] attn[d, t] ----
            ops = {}
            for kt in range(NT):
                if kt not in wo_strips:
                    w_s = stream.tile([P, D], f16, tag="wstrip", name=f"wo{kt}")
                    nc.sync.dma_start(w_s[:], Wo_d[P * kt : P * (kt + 1), :])
                    wo_strips[kt] = w_s
                w_s = wo_strips[kt]
                for m in range(NT):
                    if kt == 0 and m % 2 == 0:
                        ops[m // 2] = psum.tile([P, 1024], f32, tag="ps", name=f"ops{m//2}")
                    nc.tensor.matmul(
                        ops[m // 2][:, 512 * (m % 2) : 512 * (m % 2) + 512],
                        lhsT=w_s[:, P * m : P * (m + 1)],
                        rhs=attn_sb[:, kt, :],
                        start=(kt == 0),
                        stop=(kt == NT - 1),
                    )
            for m in range(NT):
                ot = small.tile([P, 512], f32, tag="osb", name=f"ot{m}")
                nc.scalar.activation(
                    ot[:],
                    ops[m // 2][:, 512 * (m % 2) : 512 * (m % 2) + 512],
                    AF.Identity,
                    bias=bo_sb[:, m : m + 1],
                )
                nc.sync.dma_start(outT_d[P * m : P * (m + 1), :], ot[:])

    nc.compile()
    return nc


def _get_nc():
    if "nc" not in _CACHE:
        _CACHE["nc"] = _build()
    return _CACHE["nc"]


def kernel(q_in, k_in, v_in, Wq, bq, Wk, bk, Wv, bv, Wo, bo):
    from concourse.bass_utils import run_bass_kernel_spmd

    bf = np.float16
    q_in = np.asarray(q_in, dtype=np.float32)
    k_in = np.asarray(k_in, dtype=np.float32)
    v_in = np.asarray(v_in, dtype=np.float32)
    Wq_b = np.ascontiguousarray(np.asarray(Wq, dtype=np.float32).astype(bf))
    Wk_b = np.ascontiguousarray(np.asarray(Wk, dtype=np.float32).astype(bf))
    Wv_b = np.ascontiguousarray(np.asarray(Wv, dtype=np.float32).astype(bf))
    Wo_b = np.ascontiguousarray(np.asarray(Wo, dtype=np.float32).astype(bf))
    Wo = np.asarray(Wo, dtype=np.float32)
    bq = np.asarray(bq, dtype=np.float32)
    bk = np.asarray(bk, dtype=np.float32)
    bv = np.asarray(bv, dtype=np.float32)
    bo = np.asarray(bo, dtype=np.float32)

    B = q_in.shape[0]
    # softmax rows sum to 1, so V's bias passes through attention unchanged
    # and folds into the output projection's bias.
    bo_eff = (bv @ Wo + bo).astype(np.float32)

    def pack_bias(b):
        # [D] -> [P, NT] with element (p, t) = b[128*t + p]
        return np.ascontiguousarray(b.reshape(NT, P).T)

    nc = _get_nc()

    in_maps = []
    for i in range(N_CORES):
        b, half = i // 2, i % 2
        rows = slice(SQ * half, SQ * (half + 1))
        in_maps.append(
            {
                "qT": np.ascontiguousarray(q_in[b, rows, :].T.astype(bf)),
                "kT": np.ascontiguousarray(k_in[b].T.astype(bf)),
                "vT": np.ascontiguousarray(v_in[b].T.astype(bf)),
                "Wq": Wq_b,
                "Wk": Wk_b,
                "Wv": Wv_b,
                "Wo": Wo_b,
                "bq": pack_bias(bq),
                "bk": pack_bias(bk),
                "bo_eff": pack_bias(bo_eff),
                "ones": np.ones((P, NT, H), dtype=bf),
            }
        )

    res = run_bass_kernel_spmd(
        nc, in_maps, core_ids=list(range(N_CORES)), trace=TRACE, tmpdir=TMPDIR
    )
    _CACHE["last"] = res

    out = np.empty((B, S, D), dtype=np.float32)
    for i in range(N_CORES):
        b, half = i // 2, i % 2
        out[b, SQ * half : SQ * (half + 1), :] = res.results[i]["outT"].T
    return out


# revision 30
# speedup vs baseline: 1.2218x; 1.2218x over previous
"""MultiHeadAttention (B=4, S=1024, D=1024, H=16) on 8 TRN2 NeuronCores.

Sharding (no collectives): core i handles batch b=i//2 and query-row half
i%2 (512 query tokens). K/V projections for the batch are duplicated across
the two cores sharing it; each core computes its 512 output rows fully.

Layouts on device (all "transposed"/feature-major so the d_model contraction
sits on SBUF partitions):
  qT  [1024, 512]   q_in[b, rows].T  (host-transposed, bf16)
  kT  [1024, 1024]  k_in[b].T        (f16)
  vT  [1024, 1024]  v_in[b].T        (f16)
  W*  [1024, 1024]  natural [d_in, d_out] (f16)
  outT[1024, 512]   fp32 -> host transposes back

Per-head attention with scores kept transposed (Sk on partitions, Sq free)
so softmax(P)@V needs no on-chip transposes; the softmax denominator comes
for free from a ones-column appended to each head's V block; normalization
is folded in post-V (1/denom commutes with the V matmul per query column).

Matmul operands are bf16 (fp32 weights would serialize a two-pass
LDWEIGHTS against every matmul: measured 396ns/MM vs 213ns streaming);
accumulation and the softmax denominator chain stay fp32 in PSUM.
All PSUM tiles are [128, 1024] two-bank pairs (4 bufs = all 8 banks); exps
run one [128, 1024] activation per head pair; attention PSUM is drained to
SBUF by a short DVE copy so the banks free before the normalization chain.
"""

import sys

sys.path.insert(0, "/opt/trn_rl_repo")

import numpy as np

D = 1024
H = 16
DH = 64
P = 128
SQ = 512      # query tokens per core
S = 1024      # kv tokens per core (full batch)
NT = 8        # number of 128-wide tiles along d_model
N_CORES = 8

_CACHE = {}
TRACE = False  # set True (e.g. from test.py) to capture an NTFF profile
TMPDIR = None  # where to keep NEFF/NTFF artifacts when tracing


def _build():
    import concourse.bacc as bacc
    import concourse.mybir as mybir
    import concourse.tile as tile

    f32 = mybir.dt.float32
    f16 = mybir.dt.float16
    AF = mybir.ActivationFunctionType

    nc = bacc.Bacc("TRN2", target_bir_lowering=False, debug=False, num_devices=N_CORES)

    qT_d = nc.dram_tensor("qT", [D, SQ], f16, kind="ExternalInput")
    kT_d = nc.dram_tensor("kT", [D, S], f16, kind="ExternalInput")
    vT_d = nc.dram_tensor("vT", [D, S], f16, kind="ExternalInput")
    Wq_d = nc.dram_tensor("Wq", [D, D], f16, kind="ExternalInput")
    Wk_d = nc.dram_tensor("Wk", [D, D], f16, kind="ExternalInput")
    Wv_d = nc.dram_tensor("Wv", [D, D], f16, kind="ExternalInput")
    Wo_d = nc.dram_tensor("Wo", [D, D], f16, kind="ExternalInput")
    bq_d = nc.dram_tensor("bq", [P, NT], f32, kind="ExternalInput")
    bk_d = nc.dram_tensor("bk", [P, NT], f32, kind="ExternalInput")
    bo_d = nc.dram_tensor("bo_eff", [P, NT], f32, kind="ExternalInput")
    ones_d = nc.dram_tensor("ones", [P, NT, H], f16, kind="ExternalInput")
    outT_d = nc.dram_tensor("outT", [D, SQ], f32, kind="ExternalOutput")

    with tile.TileContext(nc) as tc:
        with (
            tc.tile_pool(name="res", bufs=1) as res,
            tc.tile_pool(name="res2", bufs=1) as res2,
            tc.tile_pool(name="stream", bufs=6) as stream,
            tc.tile_pool(name="expp", bufs=6) as expp,
            tc.tile_pool(name="small", bufs=4) as small,
            tc.tile_pool(name="psum", bufs=4, space="PSUM") as psum,
        ):
            # ---- resident SBUF tensors ----
            QT_sb = res.tile([P, NT, SQ], f16, tag="QT")        # [d_part, m, sq]
            KT_sb = res.tile([P, NT, S], f16, tag="KT")         # [d_part, m, sk]
            Vaug_sb = res.tile([P, NT, H * (DH + 1)], f16, tag="Vaug")  # [sk_part, m, 16*65]
            attn_sb = res.tile([P, NT, SQ], f16, tag="attn")    # [d_part, kt, sq]
            bq_sb = res.tile([P, NT], f32, tag="bq")
            bk_sb = res.tile([P, NT], f32, tag="bk")
            bo_sb = res.tile([P, NT], f32, tag="bo")
            ones_sb = res.tile([P, NT, H], f16, tag="ones")

            # ---- Q projection: QT[128m+i, t] = sum_k Wq[k, 128m+i] qT[k, t] ----
            qps = {}
            for k in range(NT):
                w_s = stream.tile([P, D], f16, tag="wstrip")
                nc.sync.dma_start(w_s[:], Wq_d[P * k : P * (k + 1), :])
                x_s = stream.tile([P, SQ], f16, tag="xstrip")
                nc.sync.dma_start(x_s[:], qT_d[P * k : P * (k + 1), :])
                for m in range(NT):
                    if k == 0 and m % 2 == 0:
                        qps[m // 2] = psum.tile([P, 1024], f32, tag="ps", name=f"qps{m//2}")
                    nc.tensor.matmul(
                        qps[m // 2][:, 512 * (m % 2) : 512 * (m % 2) + 512],
                        lhsT=w_s[:, P * m : P * (m + 1)],
                        rhs=x_s[:],
                        start=(k == 0),
                        stop=(k == NT - 1),
                    )
            # small/scattered loads, emitted late so they don't head-block the
            # DMA queues ahead of the first weight strips
            nc.sync.dma_start(bq_sb[:], bq_d[:])
            nc.sync.dma_start(bk_sb[:], bk_d[:])
            nc.sync.dma_start(bo_sb[:], bo_d[:])
            nc.sync.dma_start(ones_sb[:], ones_d[:])
            for m in range(NT):
                nc.scalar.activation(
                    QT_sb[:, m, :],
                    qps[m // 2][:, 512 * (m % 2) : 512 * (m % 2) + 512],
                    AF.Identity,
                    bias=bq_sb[:, m : m + 1],
                )

            # ones columns of V_aug via DVE strided copy from the staged tile
            # (direct scatter-DMA costs ~16k 4B descriptors and clogs the queue)
            ones_cols = Vaug_sb[:].rearrange("p m (h c) -> p m h c", c=DH + 1)[
                :, :, :, DH
            ]
            nc.vector.tensor_copy(ones_cols, ones_sb[:])

            # ---- K projection ----
            # kT resident in SBUF; Wk streamed once as half-strips; one
            # N=1024 matmul per (k, m) into a two-bank pair tile.
            kT_res = res2.tile([P, NT, S], f16, tag="kT")
            for half in range(2):
                kps = {}
                for k in range(NT):
                    w_s = stream.tile([P, 512], f16, tag="xstrip", name=f"wkh{half}_{k}")
                    nc.sync.dma_start(
                        w_s[:], Wk_d[P * k : P * (k + 1), 512 * half : 512 * (half + 1)]
                    )
                    if half == 0:
                        nc.sync.dma_start(
                            kT_res[:, k, :], kT_d[P * k : P * (k + 1), :]
                        )
                    for mm in range(4):
                        m = 4 * half + mm
                        if k == 0:
                            kps[m] = psum.tile([P, 1024], f32, tag="ps", name=f"kps{m}")
                        for n in range(2):
                            nc.tensor.matmul(
                                kps[m][:, 512 * n : 512 * (n + 1)],
                                lhsT=w_s[:, P * mm : P * (mm + 1)],
                                rhs=kT_res[:, k, 512 * n : 512 * (n + 1)],
                                start=(k == 0),
                                stop=(k == NT - 1),
                            )
                for mm in range(4):
                    m = 4 * half + mm
                    nc.scalar.activation(
                        KT_sb[:, m, :], kps[m][:], AF.Identity, bias=bk_sb[:, m : m + 1]
                    )

            # ---- V projection (d-halves) + attention, interleaved so the
            # first four head pairs (d-half 0) start while V half 1 projects.
            def v_proj_half(n):
                vps = {}
                for k in range(NT):
                    v_s = stream.tile([P, S], f16, tag="wstrip", name=f"vs{n}_{k}")
                    nc.sync.dma_start(v_s[:], vT_d[P * k : P * (k + 1), :])
                    w_s = stream.tile([P, 512], f16, tag="xstrip", name=f"wvh{n}_{k}")
                    nc.sync.dma_start(
                        w_s[:], Wv_d[P * k : P * (k + 1), 512 * n : 512 * (n + 1)]
                    )
                    for m in range(NT):
                        if k == 0 and m % 2 == 0:
                            vps[m // 2] = psum.tile(
                                [P, 1024], f32, tag="ps", name=f"vps{n}_{m//2}"
                            )
                        nc.tensor.matmul(
                            vps[m // 2][:, 512 * (m % 2) : 512 * (m % 2) + 512],
                            lhsT=v_s[:, P * m : P * (m + 1)],
                            rhs=w_s[:],
                            start=(k == 0),
                            stop=(k == NT - 1),
                        )
                for m in range(NT):
                    dst = Vaug_sb[:, m, 520 * n : 520 * n + 520].rearrange(
                        "p (h c) -> p h c", c=DH + 1
                    )[:, :, 0:DH]
                    src = vps[m // 2][
                        :, 512 * (m % 2) : 512 * (m % 2) + 512
                    ].rearrange("p (h c) -> p h c", c=DH)
                    if m % 2 == 0:
                        nc.vector.tensor_copy(dst, src)
                    else:
                        nc.scalar.copy(dst, src)

            def attention_pair(p):
                h0, h1 = 2 * p, 2 * p + 1
                att = psum.tile([P, 1024], f32, tag="ps", name=f"att{p}")
                for s in range(NT):
                    sc = psum.tile([P, 1024], f32, tag="ps", name=f"sc{p}_{s}")
                    nc.tensor.matmul(
                        sc[:, 0:512],
                        lhsT=KT_sb[0:64, p, P * s : P * (s + 1)],
                        rhs=QT_sb[0:64, p, :],
                        start=True,
                        stop=True,
                    )
                    nc.tensor.matmul(
                        sc[:, 512:1024],
                        lhsT=KT_sb[64:128, p, P * s : P * (s + 1)],
                        rhs=QT_sb[64:128, p, :],
                        start=True,
                        stop=True,
                    )
                    e = expp.tile([P, 1024], f16, tag="exp", name=f"e{p}_{s}")
                    nc.scalar.activation(e[:], sc[:], AF.Exp, scale=0.125)
                    nc.tensor.matmul(
                        att[0 : DH + 1, 0:512],
                        lhsT=Vaug_sb[:, s, 65 * h0 : 65 * h0 + 65],
                        rhs=e[:, 0:512],
                        start=(s == 0),
                        stop=(s == NT - 1),
                    )
                    nc.tensor.matmul(
                        att[0 : DH + 1, 512:1024],
                        lhsT=Vaug_sb[:, s, 65 * h1 : 65 * h1 + 65],
                        rhs=e[:, 512:1024],
                        start=(s == 0),
                        stop=(s == NT - 1),
                    )
                # drain PSUM fast (frees both banks); normalization happens a
                # pair later (attention_norm) so this copy never queues behind
                # the previous pair's normalize chain on DVE.
                acp = small.tile([DH + 1, 1024], f32, tag="attcp", name=f"acp{p}")
                nc.vector.tensor_copy(acp[:], att[0 : DH + 1, :])
                return acp

            def attention_norm(p, acp):
                rc = small.tile([1, 1024], f32, tag="recip", name=f"rc{p}")
                nc.vector.reciprocal(rc[:], acp[DH : DH + 1, :])
                bc = small.tile([64, 1024], f32, tag="bc", name=f"bc{p}")
                nc.gpsimd.partition_broadcast(bc[:], rc[:])
                nc.vector.tensor_mul(
                    attn_sb[0:64, p, :], acp[0:DH, 0:512], bc[:, 0:512]
                )
                t1 = small.tile([64, 512], f16, tag="tmp", name=f"t1{p}")
                nc.vector.tensor_mul(t1[:], acp[0:DH, 512:1024], bc[:, 512:1024])
                nc.sync.dma_start(attn_sb[64:128, p, :], t1[:])

            pend = []  # (p, acp) pending normalization, one pair behind
            v_proj_half(0)
            for p in range(3):
                pend.append((p, attention_pair(p)))
                if len(pend) > 1:
                    attention_norm(*pend.pop(0))
            pend.append((3, attention_pair(3)))
            v_proj_half(1)  # Vaug half-1 copies go ahead of the pending norms
            for p in range(4, NT):
                attention_norm(*pend.pop(0))
                pend.append((p, attention_pair(p)))
                if len(pend) > 1:
                    attention_norm(*pend.pop(0))
            # prefetch the first output-projection weight strips while the last
            # pairs drain (only 5: a 6-slot pool fully claimed here would block
            # the sync queue ahead of the t1 DMAs the norms below still need)
            wo_strips = {}
            for kt in range(5):
                w_s = stream.tile([P, D], f16, tag="wstrip", name=f"wo{kt}")
                nc.sync.dma_start(w_s[:], Wo_d[P * kt : P * (kt + 1), :])
                wo_strips[kt] = w_s
            while pend:
                attention_norm(*pend.pop(0))

            # ---- output projection: outT[128m+i, t] = sum_d Wo[d, 128m+i] attn[d, t] ----
            ops = {}
            for kt in range(NT):
                if kt not in wo_strips:
                    w_s = stream.tile([P, D], f16, tag="wstrip", name=f"wo{kt}")
                    nc.sync.dma_start(w_s[:], Wo_d[P * kt : P * (kt + 1), :])
                    wo_strips[kt] = w_s
                w_s = wo_strips[kt]
                for m in range(NT):
                    if kt == 0 and m % 2 == 0:
                        ops[m // 2] = psum.tile([P, 1024], f32, tag="ps", name=f"ops{m//2}")
                    nc.tensor.matmul(
                        ops[m // 2][:, 512 * (m % 2) : 512 * (m % 2) + 512],
                        lhsT=w_s[:, P * m : P * (m + 1)],
                        rhs=attn_sb[:, kt, :],
                        start=(kt == 0),
                        stop=(kt == NT - 1),
                    )
            for m in range(NT):
                ot = small.tile([P, 512], f32, tag="osb", name=f"ot{m}")
                nc.scalar.activation(
                    ot[:],
                    ops[m // 2][:, 512 * (m % 2) : 512 * (m % 2) + 512],
                    AF.Identity,
                    bias=bo_sb[:, m : m + 1],
                )
                nc.sync.dma_start(outT_d[P * m : P * (m + 1), :], ot[:])

    nc.compile()
    return nc


def _get_nc():
    if "nc" not in _CACHE:
        _CACHE["nc"] = _build()
    return _CACHE["nc"]


def kernel(q_in, k_in, v_in, Wq, bq, Wk, bk, Wv, bv, Wo, bo):
    from concourse.bass_utils import run_bass_kernel_spmd

    bf = np.float16
    q_in = np.asarray(q_in, dtype=np.float32)
    k_in = np.asarray(k_in, dtype=np.float32)
    v_in = np.asarray(v_in, dtype=np.float32)
    Wq_b = np.ascontiguousarray(np.asarray(Wq, dtype=np.float32).astype(bf))
    Wk_b = np.ascontiguousarray(np.asarray(Wk, dtype=np.float32).astype(bf))
    Wv_b = np.ascontiguousarray(np.asarray(Wv, dtype=np.float32).astype(bf))
    Wo_b = np.ascontiguousarray(np.asarray(Wo, dtype=np.float32).astype(bf))
    Wo = np.asarray(Wo, dtype=np.float32)
    bq = np.asarray(bq, dtype=np.float32)
    bk = np.asarray(bk, dtype=np.float32)
    bv = np.asarray(bv, dtype=np.float32)
    bo = np.asarray(bo, dtype=np.float32)

    B = q_in.shape[0]
    # softmax rows sum to 1, so V's bias passes through attention unchanged
    # and folds into the output projection's bias.
    bo_eff = (bv @ Wo + bo).astype(np.float32)

    def pack_bias(b):
        # [D] -> [P, NT] with element (p, t) = b[128*t + p]
        return np.ascontiguousarray(b.reshape(NT, P).T)

    nc = _get_nc()

    in_maps = []
    for i in range(N_CORES):
        b, half = i // 2, i % 2
        rows = slice(SQ * half, SQ * (half + 1))
        in_maps.append(
            {
                "qT": np.ascontiguousarray(q_in[b, rows, :].T.astype(bf)),
                "kT": np.ascontiguousarray(k_in[b].T.astype(bf)),
                "vT": np.ascontiguousarray(v_in[b].T.astype(bf)),
                "Wq": Wq_b,
                "Wk": Wk_b,
                "Wv": Wv_b,
                "Wo": Wo_b,
                "bq": pack_bias(bq),
                "bk": pack_bias(bk),
                "bo_eff": pack_bias(bo_eff),
                "ones": np.ones((P, NT, H), dtype=bf),
            }
        )

    res = run_bass_kernel_spmd(
        nc, in_maps, core_ids=list(range(N_CORES)), trace=TRACE, tmpdir=TMPDIR
    )
    _CACHE["last"] = res

    out = np.empty((B, S, D), dtype=np.float32)
    for i in range(N_CORES):
        b, half = i // 2, i % 2
        out[b, SQ * half : SQ * (half + 1), :] = res.results[i]["outT"].T
    return out


# revision 32
# speedup vs baseline: 1.2591x; 1.0306x over previous
"""MultiHeadAttention (B=4, S=1024, D=1024, H=16) on 8 TRN2 NeuronCores.

Sharding (no collectives): core i handles batch b=i//2 and query-row half
i%2 (512 query tokens). K/V projections for the batch are duplicated across
the two cores sharing it; each core computes its 512 output rows fully.

Layouts on device (all "transposed"/feature-major so the d_model contraction
sits on SBUF partitions):
  qT  [1024, 512]   q_in[b, rows].T  (host-transposed, bf16)
  kT  [1024, 1024]  k_in[b].T        (f16)
  vT  [1024, 1024]  v_in[b].T        (f16)
  W*  [1024, 1024]  natural [d_in, d_out] (f16)
  outT[1024, 512]   fp32 -> host transposes back

Per-head attention with scores kept transposed (Sk on partitions, Sq free)
so softmax(P)@V needs no on-chip transposes; the softmax denominator comes
for free from a ones-column appended to each head's V block; normalization
is folded in post-V (1/denom commutes with the V matmul per query column).

Matmul operands are bf16 (fp32 weights would serialize a two-pass
LDWEIGHTS against every matmul: measured 396ns/MM vs 213ns streaming);
accumulation and the softmax denominator chain stay fp32 in PSUM.
All PSUM tiles are [128, 1024] two-bank pairs (4 bufs = all 8 banks); exps
run one [128, 1024] activation per head pair; attention PSUM is drained to
SBUF by a short DVE copy so the banks free before the normalization chain.
"""

import sys

sys.path.insert(0, "/opt/trn_rl_repo")

import numpy as np

D = 1024
H = 16
DH = 64
P = 128
SQ = 512      # query tokens per core
S = 1024      # kv tokens per core (full batch)
NT = 8        # number of 128-wide tiles along d_model
N_CORES = 8

_CACHE = {}
TRACE = False  # set True (e.g. from test.py) to capture an NTFF profile
TMPDIR = None  # where to keep NEFF/NTFF artifacts when tracing


def _build():
    import concourse.bacc as bacc
    import concourse.mybir as mybir
    import concourse.tile as tile

    f32 = mybir.dt.float32
    f16 = mybir.dt.float16
    AF = mybir.ActivationFunctionType

    nc = bacc.Bacc("TRN2", target_bir_lowering=False, debug=False, num_devices=N_CORES)

    qT_d = nc.dram_tensor("qT", [D, SQ], f16, kind="ExternalInput")
    kT_d = nc.dram_tensor("kT", [D, S], f16, kind="ExternalInput")
    vT_d = nc.dram_tensor("vT", [D, S], f16, kind="ExternalInput")
    Wq_d = nc.dram_tensor("Wq", [D, D], f16, kind="ExternalInput")
    Wk_d = nc.dram_tensor("Wk", [D, D], f16, kind="ExternalInput")
    Wv_d = nc.dram_tensor("Wv", [D, D], f16, kind="ExternalInput")
    Wo_d = nc.dram_tensor("Wo", [D, D], f16, kind="ExternalInput")
    bq_d = nc.dram_tensor("bq", [P, NT], f32, kind="ExternalInput")
    bk_d = nc.dram_tensor("bk", [P, NT], f32, kind="ExternalInput")
    bo_d = nc.dram_tensor("bo_eff", [P, NT], f32, kind="ExternalInput")
    ones_d = nc.dram_tensor("ones", [P, NT, H], f16, kind="ExternalInput")
    outT_d = nc.dram_tensor("outT", [D, SQ], f32, kind="ExternalOutput")

    with tile.TileContext(nc) as tc:
        with (
            tc.tile_pool(name="res", bufs=1) as res,
            tc.tile_pool(name="res2", bufs=1) as res2,
            tc.tile_pool(name="stream", bufs=6) as stream,
            tc.tile_pool(name="expp", bufs=6) as expp,
            tc.tile_pool(name="small", bufs=4) as small,
            tc.tile_pool(name="dpool", bufs=2, space="DRAM") as dpool,
            tc.tile_pool(name="psum", bufs=4, space="PSUM") as psum,
        ):
            # ---- resident SBUF tensors ----
            QT_sb = res.tile([P, NT, SQ], f16, tag="QT")        # [d_part, m, sq]
            KT_sb = res.tile([P, NT, S], f16, tag="KT")         # [d_part, m, sk]
            Vaug_sb = res.tile([P, NT, H * (DH + 1)], f16, tag="Vaug")  # [sk_part, m, 16*65]
            attn_sb = res.tile([P, NT, SQ], f16, tag="attn")    # [d_part, kt, sq]
            bq_sb = res.tile([P, NT], f32, tag="bq")
            bk_sb = res.tile([P, NT], f32, tag="bk")
            bo_sb = res.tile([P, NT], f32, tag="bo")
            ones_sb = res.tile([P, NT, H], f16, tag="ones")

            # ---- Q projection: QT[128m+i, t] = sum_k Wq[k, 128m+i] qT[k, t] ----
            qps = {}
            for k in range(NT):
                w_s = stream.tile([P, D], f16, tag="wstrip")
                nc.sync.dma_start(w_s[:], Wq_d[P * k : P * (k + 1), :])
                x_s = stream.tile([P, SQ], f16, tag="xstrip")
                nc.sync.dma_start(x_s[:], qT_d[P * k : P * (k + 1), :])
                for m in range(NT):
                    if k == 0 and m % 2 == 0:
                        qps[m // 2] = psum.tile([P, 1024], f32, tag="ps", name=f"qps{m//2}")
                    nc.tensor.matmul(
                        qps[m // 2][:, 512 * (m % 2) : 512 * (m % 2) + 512],
                        lhsT=w_s[:, P * m : P * (m + 1)],
                        rhs=x_s[:],
                        start=(k == 0),
                        stop=(k == NT - 1),
                    )
            # small/scattered loads, emitted late so they don't head-block the
            # DMA queues ahead of the first weight strips
            nc.sync.dma_start(bq_sb[:], bq_d[:])
            nc.sync.dma_start(bk_sb[:], bk_d[:])
            nc.sync.dma_start(bo_sb[:], bo_d[:])
            nc.sync.dma_start(ones_sb[:], ones_d[:])
            for m in range(NT):
                nc.scalar.activation(
                    QT_sb[:, m, :],
                    qps[m // 2][:, 512 * (m % 2) : 512 * (m % 2) + 512],
                    AF.Identity,
                    bias=bq_sb[:, m : m + 1],
                )

            # ones columns of V_aug via DVE strided copy from the staged tile
            # (direct scatter-DMA costs ~16k 4B descriptors and clogs the queue)
            ones_cols = Vaug_sb[:].rearrange("p m (h c) -> p m h c", c=DH + 1)[
                :, :, :, DH
            ]
            nc.vector.tensor_copy(ones_cols, ones_sb[:])

            # ---- K projection ----
            # kT resident in SBUF; Wk streamed once as half-strips; one
            # N=1024 matmul per (k, m) into a two-bank pair tile.
            kT_res = res2.tile([P, NT, S], f16, tag="kT")
            for half in range(2):
                kps = {}
                for k in range(NT):
                    w_s = stream.tile([P, 512], f16, tag="xstrip", name=f"wkh{half}_{k}")
                    nc.sync.dma_start(
                        w_s[:], Wk_d[P * k : P * (k + 1), 512 * half : 512 * (half + 1)]
                    )
                    if half == 0:
                        nc.sync.dma_start(
                            kT_res[:, k, :], kT_d[P * k : P * (k + 1), :]
                        )
                    for mm in range(4):
                        m = 4 * half + mm
                        if k == 0:
                            kps[m] = psum.tile([P, 1024], f32, tag="ps", name=f"kps{m}")
                        for n in range(2):
                            nc.tensor.matmul(
                                kps[m][:, 512 * n : 512 * (n + 1)],
                                lhsT=w_s[:, P * mm : P * (mm + 1)],
                                rhs=kT_res[:, k, 512 * n : 512 * (n + 1)],
                                start=(k == 0),
                                stop=(k == NT - 1),
                            )
                for mm in range(4):
                    m = 4 * half + mm
                    nc.scalar.activation(
                        KT_sb[:, m, :], kps[m][:], AF.Identity, bias=bk_sb[:, m : m + 1]
                    )

            # ---- V projection (d-halves) + attention, interleaved so the
            # first four head pairs (d-half 0) start while V half 1 projects.
            def v_proj_half(n):
                vps = {}
                for k in range(NT):
                    v_s = stream.tile([P, S], f16, tag="wstrip", name=f"vs{n}_{k}")
                    nc.sync.dma_start(v_s[:], vT_d[P * k : P * (k + 1), :])
                    w_s = stream.tile([P, 512], f16, tag="xstrip", name=f"wvh{n}_{k}")
                    nc.sync.dma_start(
                        w_s[:], Wv_d[P * k : P * (k + 1), 512 * n : 512 * (n + 1)]
                    )
                    for m in range(NT):
                        if k == 0 and m % 2 == 0:
                            vps[m // 2] = psum.tile(
                                [P, 1024], f32, tag="ps", name=f"vps{n}_{m//2}"
                            )
                        nc.tensor.matmul(
                            vps[m // 2][:, 512 * (m % 2) : 512 * (m % 2) + 512],
                            lhsT=v_s[:, P * m : P * (m + 1)],
                            rhs=w_s[:],
                            start=(k == 0),
                            stop=(k == NT - 1),
                        )
                for m in range(NT):
                    dst = Vaug_sb[:, m, 520 * n : 520 * n + 520].rearrange(
                        "p (h c) -> p h c", c=DH + 1
                    )[:, :, 0:DH]
                    src = vps[m // 2][
                        :, 512 * (m % 2) : 512 * (m % 2) + 512
                    ].rearrange("p (h c) -> p h c", c=DH)
                    if m % 2 == 0:
                        nc.vector.tensor_copy(dst, src)
                    else:
                        nc.scalar.copy(dst, src)

            def attention_pair(p):
                h0, h1 = 2 * p, 2 * p + 1
                att = psum.tile([P, 1024], f32, tag="ps", name=f"att{p}")
                for s in range(NT):
                    sc = psum.tile([P, 1024], f32, tag="ps", name=f"sc{p}_{s}")
                    nc.tensor.matmul(
                        sc[:, 0:512],
                        lhsT=KT_sb[0:64, p, P * s : P * (s + 1)],
                        rhs=QT_sb[0:64, p, :],
                        start=True,
                        stop=True,
                    )
                    nc.tensor.matmul(
                        sc[:, 512:1024],
                        lhsT=KT_sb[64:128, p, P * s : P * (s + 1)],
                        rhs=QT_sb[64:128, p, :],
                        start=True,
                        stop=True,
                    )
                    e = expp.tile([P, 1024], f16, tag="exp", name=f"e{p}_{s}")
                    nc.scalar.activation(e[:], sc[:], AF.Exp, scale=0.125)
                    nc.tensor.matmul(
                        att[0 : DH + 1, 0:512],
                        lhsT=Vaug_sb[:, s, 65 * h0 : 65 * h0 + 65],
                        rhs=e[:, 0:512],
                        start=(s == 0),
                        stop=(s == NT - 1),
                    )
                    nc.tensor.matmul(
                        att[0 : DH + 1, 512:1024],
                        lhsT=Vaug_sb[:, s, 65 * h1 : 65 * h1 + 65],
                        rhs=e[:, 512:1024],
                        start=(s == 0),
                        stop=(s == NT - 1),
                    )
                # drain PSUM fast (frees both banks); normalization happens a
                # pair later (attention_norm) so this copy never queues behind
                # the previous pair's normalize chain on DVE.
                acp = small.tile([DH + 1, 1024], f32, tag="attcp", name=f"acp{p}")
                nc.vector.tensor_copy(acp[:], att[0 : DH + 1, :])
                return acp

            def attention_norm(p, acp):
                # DVE reciprocal cost scales with free-size per lane (a
                # [1,1024] recip measured 6.5us), so bounce the denominator
                # row through DRAM to respread it over 64 lanes (0.25us),
                # then broadcast back with a step-0 DRAM read.
                d1 = dpool.tile([1, 1024], f32, tag="d1", name=f"d1_{p}")
                nc.sync.dma_start(d1[:], acp[DH : DH + 1, :])
                rsh = small.tile([64, 16], f32, tag="rsh", name=f"rsh{p}")
                nc.sync.dma_start(
                    rsh[:], d1[:].rearrange("o (a b) -> (o a) b", a=64)
                )
                rr = small.tile([64, 16], f32, tag="rr", name=f"rr{p}")
                nc.vector.reciprocal(rr[:], rsh[:])
                d2 = dpool.tile([1, 1024], f32, tag="d2", name=f"d2_{p}")
                nc.sync.dma_start(
                    d2[:].rearrange("o (a b) -> (o a) b", a=64), rr[:]
                )
                bc = small.tile([64, 1024], f32, tag="bc", name=f"bc{p}")
                nc.sync.dma_start(bc[:], d2[:].to_broadcast([64, 1024]))
                nc.vector.tensor_mul(
                    attn_sb[0:64, p, :], acp[0:DH, 0:512], bc[:, 0:512]
                )
                t1 = small.tile([64, 512], f16, tag="tmp", name=f"t1{p}")
                nc.vector.tensor_mul(t1[:], acp[0:DH, 512:1024], bc[:, 512:1024])
                nc.sync.dma_start(attn_sb[64:128, p, :], t1[:])

            pend = []  # (p, acp) pending normalization, one pair behind
            v_proj_half(0)
            for p in range(3):
                pend.append((p, attention_pair(p)))
                if len(pend) > 1:
                    attention_norm(*pend.pop(0))
            pend.append((3, attention_pair(3)))
            v_proj_half(1)  # Vaug half-1 copies go ahead of the pending norms
            for p in range(4, NT):
                attention_norm(*pend.pop(0))
                pend.append((p, attention_pair(p)))
                if len(pend) > 1:
                    attention_norm(*pend.pop(0))
            # prefetch the first output-projection weight strips while the last
            # pairs drain (only 5: a 6-slot pool fully claimed here would block
            # the sync queue ahead of the t1 DMAs the norms below still need)
            wo_strips = {}
            for kt in range(5):
                w_s = stream.tile([P, D], f16, tag="wstrip", name=f"wo{kt}")
                nc.sync.dma_start(w_s[:], Wo_d[P * kt : P * (kt + 1), :])
                wo_strips[kt] = w_s
            while pend:
                attention_norm(*pend.pop(0))

            # ---- output projection: outT[128m+i, t] = sum_d Wo[d, 128m+i] attn[d, t] ----
            ops = {}
            for kt in range(NT):
                if kt not in wo_strips:
                    w_s = stream.tile([P, D], f16, tag="wstrip", name=f"wo{kt}")
                    nc.sync.dma_start(w_s[:], Wo_d[P * kt : P * (kt + 1), :])
                    wo_strips[kt] = w_s
                w_s = wo_strips[kt]
                for m in range(NT):
                    if kt == 0 and m % 2 == 0:
                        ops[m // 2] = psum.tile([P, 1024], f32, tag="ps", name=f"ops{m//2}")
                    nc.tensor.matmul(
                        ops[m // 2][:, 512 * (m % 2) : 512 * (m % 2) + 512],
                        lhsT=w_s[:, P * m : P * (m + 1)],
                        rhs=attn_sb[:, kt, :],
                        start=(kt == 0),
                        stop=(kt == NT - 1),
                    )
            for m in range(NT):
                ot = small.tile([P, 512], f32, tag="osb", name=f"ot{m}")
                nc.scalar.activation(
                    ot[:],
                    ops[m // 2][:, 512 * (m % 2) : 512 * (m % 2) + 512],
                    AF.Identity,
                    bias=bo_sb[:, m : m + 1],
                )
                nc.sync.dma_start(outT_d[P * m : P * (m + 1), :], ot[:])

    nc.compile()
    return nc


def _get_nc():
    if "nc" not in _CACHE:
        _CACHE["nc"] = _build()
    return _CACHE["nc"]


def kernel(q_in, k_in, v_in, Wq, bq, Wk, bk, Wv, bv, Wo, bo):
    from concourse.bass_utils import run_bass_kernel_spmd

    bf = np.float16
    q_in = np.asarray(q_in, dtype=np.float32)
    k_in = np.asarray(k_in, dtype=np.float32)
    v_in = np.asarray(v_in, dtype=np.float32)
    Wq_b = np.ascontiguousarray(np.asarray(Wq, dtype=np.float32).astype(bf))
    Wk_b = np.ascontiguousarray(np.asarray(Wk, dtype=np.float32).astype(bf))
    Wv_b = np.ascontiguousarray(np.asarray(Wv, dtype=np.float32).astype(bf))
    Wo_b = np.ascontiguousarray(np.asarray(Wo, dtype=np.float32).astype(bf))
    Wo = np.asarray(Wo, dtype=np.float32)
    bq = np.asarray(bq, dtype=np.float32)
    bk = np.asarray(bk, dtype=np.float32)
    bv = np.asarray(bv, dtype=np.float32)
    bo = np.asarray(bo, dtype=np.float32)

    B = q_in.shape[0]
    # softmax rows sum to 1, so V's bias passes through attention unchanged
    # and folds into the output projection's bias.
    bo_eff = (bv @ Wo + bo).astype(np.float32)

    def pack_bias(b):
        # [D] -> [P, NT] with element (p, t) = b[128*t + p]
        return np.ascontiguousarray(b.reshape(NT, P).T)

    nc = _get_nc()

    in_maps = []
    for i in range(N_CORES):
        b, half = i // 2, i % 2
        rows = slice(SQ * half, SQ * (half + 1))
        in_maps.append(
            {
                "qT": np.ascontiguousarray(q_in[b, rows, :].T.astype(bf)),
                "kT": np.ascontiguousarray(k_in[b].T.astype(bf)),
                "vT": np.ascontiguousarray(v_in[b].T.astype(bf)),
                "Wq": Wq_b,
                "Wk": Wk_b,
                "Wv": Wv_b,
                "Wo": Wo_b,
                "bq": pack_bias(bq),
                "bk": pack_bias(bk),
                "bo_eff": pack_bias(bo_eff),
                "ones": np.ones((P, NT, H), dtype=bf),
            }
        )

    res = run_bass_kernel_spmd(
        nc, in_maps, core_ids=list(range(N_CORES)), trace=TRACE, tmpdir=TMPDIR
    )
    _CACHE["last"] = res

    out = np.empty((B, S, D), dtype=np.float32)
    for i in range(N_CORES):
        b, half = i // 2, i % 2
        out[b, SQ * half : SQ * (half + 1), :] = res.results[i]["outT"].T
    return out
